# revision 1
# baseline (speedup 1.0000x reference)
"""Paged KV-cache decode attention with ALiBi (Baichuan-style), fused
QKV + attention + output projection, tensor-parallel over heads across
8 Trainium2 NeuronCores.

Layout strategy (per core, 5 heads):
  - qT/kT computed as [640, 4] (head-dim on partitions) so scores matmuls
    need no transposes and the K-cache new-token scatter is a same-partition
    SBUF copy.
  - v computed as [4, 640] (natural) so the V new-token scatter is a tiny
    SBUF->SBUF DMA row write.
  - K cache staged host-side per core as [5, 4, 128(d), 2048(t)] (K^T),
    V cache as [5, 4, 128(t%128), 16(chunk), 128(d)] so every device DMA is
    a large (>=0.5-1MB) mostly-contiguous transfer.
  - softmax without max-subtraction (scores are O(10); exp is safe in fp32),
    masking baked into a host-precomputed additive bias (-1e30).
  - o_proj computed transposed (out^T [5120, 4]) per core; host sums the 8
    partial products (the "all-reduce").
"""

import math
import os
import sys
from contextlib import ExitStack

import numpy as np

sys.path.insert(0, "/opt/trn_rl_repo")

B = 4
E = 5120
H = 40
D = 128
BS = 16
NB = 512
MB = 128
S = MB * BS  # 2048
NCORES = 8
HPC = H // NCORES   # 5 heads per core
EPC = HPC * D       # 640

NEG = -1.0e30


def _alibi_slopes(num_heads):
    cp2 = 2 ** int(math.floor(math.log2(num_heads)))
    base = 2.0 ** (-(2.0 ** (-(math.log2(cp2) - 3))))
    slopes = base ** np.arange(1, cp2 + 1, dtype=np.float64)
    if cp2 != num_heads:
        extra_base = 2.0 ** (-(2.0 ** (-(math.log2(2 * cp2) - 3))))
        n_rem = min(cp2, num_heads - cp2)
        extra = extra_base ** np.arange(1, 1 + 2 * n_rem, 2, dtype=np.float64)
        slopes = np.concatenate([slopes, extra])
    return slopes.astype(np.float32)


_PROGRAM_CACHE = {}
LAST_RESULTS = None  # BassKernelResults of the most recent run (for test.py)


def _build_program(pos, nch):
    """Build the SPMD Bass program. pos/nch are per-sequence tuples, baked
    statically (same for all cores; per-core data varies only via inputs)."""
    import concourse.bacc as bacc
    import concourse.bass as bass
    import concourse.tile as tile
    from concourse import mybir

    f32 = mybir.dt.float32
    nc = bacc.Bacc()

    hT = nc.declare_dram_parameter("hT", [128, 40 * B], f32, isOutput=False)
    qkvw = nc.declare_dram_parameter("qkvw", [3, E, EPC], f32, isOutput=False)
    ow = nc.declare_dram_parameter("ow", [EPC, E], f32, isOutput=False)
    kt = nc.declare_dram_parameter("kt", [HPC, B, D, S], f32, isOutput=False)
    vt = nc.declare_dram_parameter("vt", [HPC, B, 128, 16, D], f32, isOutput=False)
    bias = nc.declare_dram_parameter("bias", [128, B * HPC * 16], f32, isOutput=False)
    outT = nc.declare_dram_parameter("outT", [128, 40 * B], f32, isOutput=True)

    with tile.TileContext(nc) as tc, ExitStack() as ctx:
        consts = ctx.enter_context(tc.tile_pool(name="consts", bufs=1))
        wpool = ctx.enter_context(tc.tile_pool(name="wpool", bufs=2))
        kvpool = ctx.enter_context(tc.tile_pool(name="kvpool", bufs=3))
        tmp = ctx.enter_context(tc.tile_pool(name="tmp", bufs=3))
        opool = ctx.enter_context(tc.tile_pool(name="opool", bufs=2))
        psum = ctx.enter_context(tc.tile_pool(name="psum", bufs=8, space="PSUM"))

        # ---- constants / small inputs ----
        hT_sb = consts.tile([128, 40 * B], f32)          # (E%128, (Echunk, b))
        nc.gpsimd.dma_start(out=hT_sb[:], in_=hT[:])
        bias_sb = consts.tile([128, B * HPC * 16], f32)  # (t%128, (b, h, chunk))
        nc.gpsimd.dma_start(out=bias_sb[:], in_=bias[:])
        ones_col = consts.tile([128, 1], f32)
        nc.vector.memset(ones_col[:], 1.0)
        ones_row = consts.tile([1, 128], f32)
        nc.vector.memset(ones_row[:], 1.0)

        qT_sb = consts.tile([128, HPC * B], f32)   # col = h*B + b ; partition = d
        kT_sb = consts.tile([128, HPC * B], f32)
        v_sb = consts.tile([B, EPC], f32)          # natural v rows
        colsum_sb = consts.tile([128, HPC * B], f32)
        aoT_sb = consts.tile([128, HPC * B], f32)  # unnormalized attn@V ^T
        outT_sb = consts.tile([128, 40 * B], f32)

        # ---- fused QKV projection ----
        # q,k transposed orientation: psum[oc] [128, B] accumulated over 40
        # E-chunks; lhsT = W chunk [128(E), 128(outcol)], rhs = hT chunk [128(E), B].
        for w in range(2):  # 0=q (pre-scaled on host), 1=k
            dst = qT_sb if w == 0 else kT_sb
            ps = [psum.tile([128, B], f32, tag="ps", name=f"ps_qk{w}_{i}") for i in range(HPC)]
            for g in range(10):  # groups of 4 E-chunks
                wt = wpool.tile([128, 4 * EPC], f32, tag="w")
                nc.gpsimd.dma_start(
                    out=wt[:],
                    in_=qkvw[w, g * 512:(g + 1) * 512, :].rearrange(
                        "(kl p) c -> p kl c", p=128
                    ),
                )
                for oc in range(HPC):
                    for kl in range(4):
                        kc = g * 4 + kl
                        nc.tensor.matmul(
                            ps[oc][:],
                            lhsT=wt[:, kl * EPC + oc * 128: kl * EPC + (oc + 1) * 128],
                            rhs=hT_sb[:, kc * B:(kc + 1) * B],
                            start=(kc == 0),
                            stop=(kc == 39),
                        )
            for oc in range(HPC):
                nc.scalar.copy(dst[:, oc * B:(oc + 1) * B], ps[oc][:])

        # v natural orientation: psum [B, 640] (two banks: 512 + 128),
        # lhsT = hT chunk [128(E), B], rhs = Wv chunk [128(E), 640].
        v_ps0 = psum.tile([B, 512], f32, tag="ps")
        v_ps1 = psum.tile([B, EPC - 512], f32, tag="ps")
        for g in range(10):
            wt = wpool.tile([128, 4 * EPC], f32, tag="w")
            nc.gpsimd.dma_start(
                out=wt[:],
                in_=qkvw[2, g * 512:(g + 1) * 512, :].rearrange(
                    "(kl p) c -> p kl c", p=128
                ),
            )
            for kl in range(4):
                kc = g * 4 + kl
                nc.tensor.matmul(
                    v_ps0[:],
                    lhsT=hT_sb[:, kc * B:(kc + 1) * B],
                    rhs=wt[:, kl * EPC: kl * EPC + 512],
                    start=(kc == 0),
                    stop=(kc == 39),
                )
                nc.tensor.matmul(
                    v_ps1[:],
                    lhsT=hT_sb[:, kc * B:(kc + 1) * B],
                    rhs=wt[:, kl * EPC + 512: kl * EPC + EPC],
                    start=(kc == 0),
                    stop=(kc == 39),
                )
        nc.scalar.copy(v_sb[:, :512], v_ps0[:])
        nc.scalar.copy(v_sb[:, 512:], v_ps1[:])

        # ---- attention per (b, h) ----
        for b in range(B):
            n = nch[b]
            sd = n * 128
            p = pos[b]
            for h in range(HPC):
                col = h * B + b
                Kt = kvpool.tile([128, S], f32, tag="K")
                nc.gpsimd.dma_start(out=Kt[:, :sd], in_=kt[h, b, :, :sd])
                Vt = kvpool.tile([128, 16, D], f32, tag="V")
                nc.gpsimd.dma_start(out=Vt[:, :n, :], in_=vt[h, b, :, :n, :])

                # scatter the new token K column (same partitions: d)
                nc.vector.tensor_copy(Kt[:, p:p + 1], kT_sb[:, col:col + 1])
                # scatter the new token V row (cross-partition -> DMA)
                nc.gpsimd.dma_start(
                    out=Vt[p % 128:p % 128 + 1, p // 128, :],
                    in_=v_sb[b:b + 1, h * D:(h + 1) * D],
                )

                sc_ps = psum.tile([128, 16], f32, tag="ps")
                for c in range(n):
                    nc.tensor.matmul(
                        sc_ps[:, c:c + 1],
                        lhsT=Kt[:, c * 128:(c + 1) * 128],
                        rhs=qT_sb[:, col:col + 1],
                        start=True,
                        stop=True,
                    )
                s_sb = tmp.tile([128, 16], f32, tag="s")
                nc.vector.tensor_add(
                    s_sb[:, :n],
                    sc_ps[:, :n],
                    bias_sb[:, (b * HPC + h) * 16:(b * HPC + h) * 16 + n],
                )
                attn_sb = tmp.tile([128, 16], f32, tag="attn")
                nc.scalar.activation(
                    attn_sb[:, :n],
                    s_sb[:, :n],
                    func=mybir.ActivationFunctionType.Exp,
                    accum_out=colsum_sb[:, col:col + 1],
                )
                ao_ps = psum.tile([128, 1], f32, tag="ps")
                for c in range(n):
                    nc.tensor.matmul(
                        ao_ps[:],
                        lhsT=Vt[:, c, :],
                        rhs=attn_sb[:, c:c + 1],
                        start=(c == 0),
                        stop=(c == n - 1),
                    )
                nc.scalar.copy(aoT_sb[:, col:col + 1], ao_ps[:])

        # ---- softmax normalization (batched over all 20 (b,h)) ----
        sums_ps = psum.tile([1, HPC * B], f32, tag="ps")
        nc.tensor.matmul(
            sums_ps[:], lhsT=ones_col[:], rhs=colsum_sb[:], start=True, stop=True
        )
        recip_sb = tmp.tile([1, HPC * B], f32, tag="recip")
        nc.vector.reciprocal(recip_sb[:], sums_ps[:])
        rb_ps = psum.tile([128, HPC * B], f32, tag="ps")
        nc.tensor.matmul(
            rb_ps[:], lhsT=ones_row[:], rhs=recip_sb[:], start=True, stop=True
        )
        recip_b = tmp.tile([128, HPC * B], f32, tag="recipb")
        nc.vector.tensor_copy(recip_b[:], rb_ps[:])
        attn_nT = consts.tile([128, HPC * B], f32)
        nc.vector.tensor_mul(attn_nT[:], aoT_sb[:], recip_b[:])

        # ---- output projection (transposed): outT[oc*128+p, b] ----
        # lhsT = o chunk [128(hd), 128(oc)], rhs = attn_nT slice [128(hd), B]
        for jg in range(5):  # groups of 8 outcol chunks (1024 cols)
            ops = [psum.tile([128, B], f32, tag="ps", name=f"ps_o{jg}_{i}") for i in range(8)]
            for h in range(HPC):
                ot = opool.tile([128, 1024], f32, tag="ot")
                nc.gpsimd.dma_start(
                    out=ot[:],
                    in_=ow[h * 128:(h + 1) * 128, jg * 1024:(jg + 1) * 1024],
                )
                for oc in range(8):
                    nc.tensor.matmul(
                        ops[oc][:],
                        lhsT=ot[:, oc * 128:(oc + 1) * 128],
                        rhs=attn_nT[:, h * B:(h + 1) * B],
                        start=(h == 0),
                        stop=(h == HPC - 1),
                    )
            for oc in range(8):
                g_oc = jg * 8 + oc
                nc.scalar.copy(outT_sb[:, g_oc * B:(g_oc + 1) * B], ops[oc][:])

        nc.gpsimd.dma_start(out=outT[:], in_=outT_sb[:])

    nc.compile()  # Bacc finalize: splits multi-waits (matmul 1-wait limit)
    return nc


def _prepare_core_inputs(core, hidden, qkv_w, o_w, k_cache, v_cache, bt, sl, pos):
    hs = slice(core * HPC, (core + 1) * HPC)
    es = slice(core * EPC, (core + 1) * EPC)

    qkvw = np.ascontiguousarray(qkv_w[:, :, es])
    qkvw[0] *= np.float32(D ** -0.5)

    kg = k_cache[:, hs]  # [NB, HPC, BS, D]
    vg = v_cache[:, hs]
    kt = np.empty((HPC, B, D, S), np.float32)
    vt = np.empty((HPC, B, 128, 16, D), np.float32)
    for b in range(B):
        kk = kg[bt[b]].transpose(1, 0, 2, 3).reshape(HPC, S, D)
        kt[:, b] = kk.transpose(0, 2, 1)
        vv = vg[bt[b]].transpose(1, 0, 2, 3).reshape(HPC, S, D)
        vt[:, b] = vv.reshape(HPC, 16, 128, D).transpose(0, 2, 1, 3)

    slopes = _alibi_slopes(H)[core * HPC:(core + 1) * HPC]
    t_in = np.arange(128)[:, None]
    tg = (np.arange(16)[None, :] * 128 + t_in).astype(np.float32)  # [128, 16]
    bias = np.empty((128, B, HPC, 16), np.float32)
    for b in range(B):
        for h in range(HPC):
            val = slopes[h] * (tg - np.float32(pos[b]))
            val[tg >= sl[b]] = NEG
            bias[:, b, h, :] = val

    hTf = np.ascontiguousarray(
        hidden.T.reshape(40, 128, B).transpose(1, 0, 2).reshape(128, 40 * B)
    )

    return dict(
        hT=hTf,
        qkvw=qkvw,
        ow=np.ascontiguousarray(o_w[es, :]),
        kt=kt,
        vt=vt,
        bias=np.ascontiguousarray(bias.reshape(128, B * HPC * 16)),
    )


def kernel(**inputs):
    global LAST_RESULTS
    hidden = np.asarray(inputs["hidden_states"], np.float32)
    qkv_w = np.asarray(inputs["qkv_weight"], np.float32)
    o_w = np.asarray(inputs["o_proj_weight"], np.float32)
    k_cache = np.asarray(inputs["k_cache"], np.float32)
    v_cache = np.asarray(inputs["v_cache"], np.float32)
    bt = np.asarray(inputs["block_tables"]).astype(np.int64)
    sl = np.asarray(inputs["sequence_lengths"]).astype(np.int64)

    pos = tuple(int(x) - 1 for x in sl)
    nch = tuple(int(math.ceil(int(x) / 128)) for x in sl)

    in_maps = [
        _prepare_core_inputs(c, hidden, qkv_w, o_w, k_cache, v_cache, bt, sl, pos)
        for c in range(NCORES)
    ]

    key = (pos, nch)
    if key not in _PROGRAM_CACHE:
        _PROGRAM_CACHE[key] = _build_program(pos, nch)
    nc = _PROGRAM_CACHE[key]

    from concourse.bass_utils import run_bass_kernel_spmd

    res = run_bass_kernel_spmd(
        nc,
        in_maps,
        core_ids=list(range(NCORES)),
        trace=bool(os.environ.get("BASS_TRACE")),
    )
    LAST_RESULTS = res

    out = np.zeros((B, E), np.float64)
    for c in range(NCORES):
        r = np.asarray(res.results[c]["outT"])
        out += r.reshape(128, 40, B).transpose(2, 1, 0).reshape(B, E).astype(np.float64)
    return out.astype(np.float32)



# revision 11
# speedup vs baseline: 1.9511x; 1.9511x over previous
"""Paged KV-cache decode attention with ALiBi (Baichuan-style), fused
QKV + attention + output projection, tensor-parallel over heads across
8 Trainium2 NeuronCores.

v2 design (bf16, long-moving-dim matmuls):
  - All matmul operands bf16 (1 cycle/moving-row vs 4 for fp32); PSUM
    accumulation stays fp32. Halves DMA bytes vs fp32.
  - Matmuls are oriented so the STATIONARY operand is tiny (1-20 cols)
    and the MOVING operand streams ~512 columns per instruction:
      * QKV:   out[4,1920] = x @ Wcat, stationary = xT chunk [128,4]
      * scores: out[1,512] rows (PE psum writes must start at a
        quadrant base, so rows are re-packed to [5,512] per sequence
        via basis-vector matmuls in float32r)
      * attn@V: batched over the 5 heads of one sequence via a
        [128(t),5] stationary of transposed probs against a
        [128(t), 5*128(h,d)] V chunk (block-diagonal extraction)
      * o_proj: out[4,5120], stationary = attn_out^T chunk [128,4]
  - ALiBi bias decomposed: slope_h*(t-pos_b) = slope_h*t (shared
    [5,2048] tensor add) + (-slope_h*pos_b) (per-partition scalar in
    the Exp activation). Масking via static memsets at seq_len.
  - Small transposes (q/k/attn/attn_out) on the PE with an identity
    stationary; softmax row sums via activation accum_out;
    normalization folded into the psum->sbuf copy (per-partition scale).
  - K cache staged host-side as one [128(d), 20*2048] bf16 image per
    core (single 10.5MB DMA); V as [128(t%128), 16, 640] per sequence.
"""

import math
import os
import sys
from contextlib import ExitStack

import numpy as np
import ml_dtypes

sys.path.insert(0, "/opt/trn_rl_repo")

BF16 = ml_dtypes.bfloat16

B = 4
E = 5120
H = 40
D = 128
BS = 16
NB = 512
MB = 128
S = MB * BS  # 2048
NCORES = 8
HPC = H // NCORES   # 5 heads per core
EPC = HPC * D       # 640
NKC = E // 128      # 40 contraction chunks
NQKV = 3 * EPC      # 1920 qkv output columns per core
NEG = -1.0e30


def _alibi_slopes(num_heads):
    cp2 = 2 ** int(math.floor(math.log2(num_heads)))
    base = 2.0 ** (-(2.0 ** (-(math.log2(cp2) - 3))))
    slopes = base ** np.arange(1, cp2 + 1, dtype=np.float64)
    if cp2 != num_heads:
        extra_base = 2.0 ** (-(2.0 ** (-(math.log2(2 * cp2) - 3))))
        n_rem = min(cp2, num_heads - cp2)
        extra = extra_base ** np.arange(1, 1 + 2 * n_rem, 2, dtype=np.float64)
        slopes = np.concatenate([slopes, extra])
    return slopes.astype(np.float32)


_PROGRAM_CACHE = {}
LAST_RESULTS = None  # BassKernelResults of the most recent run (for test.py)


def _build_program(pos):
    import concourse.bacc as bacc
    import concourse.bass as bass
    import concourse.tile as tile
    from concourse import mybir

    f32 = mybir.dt.float32
    f32r = mybir.dt.float32r
    bf16 = mybir.dt.bfloat16
    nc = bacc.Bacc()
    sl = tuple(p + 1 for p in pos)

    hT = nc.declare_dram_parameter("hT", [128, NKC * B], bf16, isOutput=False)
    wcat = nc.declare_dram_parameter("wcat", [128, NKC, NQKV], bf16, isOutput=False)
    kt = nc.declare_dram_parameter("kt", [128, HPC * B * S], bf16, isOutput=False)
    vt = nc.declare_dram_parameter("vt", [B, 128, 16, EPC], bf16, isOutput=False)
    wo = nc.declare_dram_parameter("wo", [128, HPC, E], bf16, isOutput=False)
    term1 = nc.declare_dram_parameter("term1", [HPC, S], f32, isOutput=False)
    term2 = nc.declare_dram_parameter("term2", [HPC, B], f32, isOutput=False)
    basis = nc.declare_dram_parameter("basis", [1, HPC * HPC], f32r, isOutput=False)
    ident = nc.declare_dram_parameter("ident", [20, 20], bf16, isOutput=False)
    outp = nc.declare_dram_parameter("outp", [B, E], bf16, isOutput=True)

    with tile.TileContext(nc) as tc, ExitStack() as ctx:
        consts = ctx.enter_context(tc.tile_pool(name="consts", bufs=1))
        ktpool = ctx.enter_context(tc.tile_pool(name="ktpool", bufs=1))
        wpool = ctx.enter_context(tc.tile_pool(name="wpool", bufs=2))
        vpool = ctx.enter_context(tc.tile_pool(name="vpool", bufs=2))
        wopool = ctx.enter_context(tc.tile_pool(name="wopool", bufs=2))
        srpool = ctx.enter_context(tc.tile_pool(name="srpool", bufs=3))
        sfpool = ctx.enter_context(tc.tile_pool(name="sfpool", bufs=2))
        psum = ctx.enter_context(tc.tile_pool(name="psum", bufs=8, space="PSUM"))

        # ---- persistent tiles ----
        hT_sb = consts.tile([128, NKC * B], bf16)
        nc.scalar.dma_start(out=hT_sb[:], in_=hT[:])
        ident_sb = consts.tile([20, 20], bf16)
        nc.scalar.dma_start(out=ident_sb[:], in_=ident[:])
        term1_sb = consts.tile([HPC, S], f32)
        nc.scalar.dma_start(out=term1_sb[:], in_=term1[:])
        term2_sb = consts.tile([HPC, B], f32)
        nc.scalar.dma_start(out=term2_sb[:], in_=term2[:])
        basis_sb = consts.tile([1, HPC * HPC], f32r)
        nc.scalar.dma_start(out=basis_sb[:], in_=basis[:])

        kt_sb = ktpool.tile([128, HPC * B * S], bf16)
        nc.sync.dma_start(out=kt_sb[:], in_=kt[:])

        vt_tiles = [None] * B
        for b in range(2):
            vt_tiles[b] = vpool.tile([128, 16, EPC], bf16, tag="V", name=f"vt{b}")
            nc.sync.dma_start(out=vt_tiles[b][:], in_=vt[b])

        qkv_sb = consts.tile([B, NQKV], bf16)
        qT_sb = consts.tile([128, HPC * B], bf16)   # col = h*B + b
        kT_sb = consts.tile([128, HPC * B], bf16)
        attn_sb = consts.tile([HPC, B * S], bf16)   # [h, b*S + t]
        attnT_sb = consts.tile([128, 16 * HPC * B], bf16)  # col = c*20 + b*5 + h
        sums_sb = [consts.tile([HPC, 4], f32, name=f"sums{b}") for b in range(B)]
        sum2_sb = [consts.tile([HPC, 2], f32, name=f"sum2{b}") for b in range(B)]
        sumt_sb = [consts.tile([HPC, 1], f32, name=f"sumt{b}") for b in range(B)]
        recip_sb = [consts.tile([HPC, 1], f32, name=f"recip{b}") for b in range(B)]
        ao_sb = [consts.tile([HPC, EPC], bf16, name=f"ao{b}") for b in range(B)]
        aoT_sb = consts.tile([128, HPC * B], bf16)  # col = h*B + b
        out_sb = consts.tile([B, E], bf16)

        # ---- fused QKV projection: qkv[4, 1920] ----
        qkv_ps = [
            psum.tile([B, min(512, NQKV - nt * 512)], f32, tag="ps", name=f"qkv_ps{nt}")
            for nt in range(4)
        ]
        for g in range(NKC // 2):
            wt = wpool.tile([128, 2 * NQKV], bf16, tag="w")
            nc.scalar.dma_start(out=wt[:], in_=wcat[:, 2 * g:2 * g + 2, :])
            for kl in range(2):
                kc = 2 * g + kl
                for nt in range(4):
                    w = min(512, NQKV - nt * 512)
                    nc.tensor.matmul(
                        qkv_ps[nt][:],
                        lhsT=hT_sb[:, kc * B:(kc + 1) * B],
                        rhs=wt[:, kl * NQKV + nt * 512: kl * NQKV + nt * 512 + w],
                        start=(kc == 0),
                        stop=(kc == NKC - 1),
                    )
        for nt in range(4):
            w = min(512, NQKV - nt * 512)
            nc.scalar.copy(qkv_sb[:, nt * 512: nt * 512 + w], qkv_ps[nt][:])

        # ---- transpose q,k: [4, 640] -> [128, 20] (col h*B+b) ----
        for w_i, dst in ((0, qT_sb), (1, kT_sb)):
            for h in range(HPC):
                tq = psum.tile([128, B], bf16, tag="ps", name=f"tq{w_i}_{h}")
                nc.tensor.transpose(
                    tq[:],
                    qkv_sb[:, w_i * EPC + h * 128: w_i * EPC + (h + 1) * 128],
                    ident_sb[:B, :B],
                )
                nc.vector.tensor_copy(dst[:, h * B:(h + 1) * B], tq[:])

        # ---- scatter new-token K column / V row ----
        for h in range(HPC):
            for b in range(B):
                col = h * B + b
                nc.vector.tensor_copy(
                    kt_sb[:, col * S + pos[b]: col * S + pos[b] + 1],
                    kT_sb[:, col:col + 1],
                )
        for b in range(2):
            nc.gpsimd.dma_start(
                out=vt_tiles[b][pos[b] % 128: pos[b] % 128 + 1, pos[b] // 128, :],
                in_=qkv_sb[b:b + 1, 2 * EPC:3 * EPC],
            )

        # ---- scores + softmax ----
        # Per (b, nt): 5 head-rows [1,512] from PE (must start at
        # partition 0), re-packed onto partitions 0..4 via basis-vector
        # matmuls (f32r streams at full rate), then bias add + Exp.
        for nt in range(4):
            lo = nt * 512
            for b in range(B):
                sp = psum.tile([HPC, 512], f32, tag="ps", name=f"sp{nt}_{b}")
                if sl[b] > lo:
                    for h in range(HPC):
                        col = h * B + b
                        s_ps = psum.tile([1, 512], f32, tag="ps", name=f"s{nt}_{b}_{h}")
                        nc.tensor.matmul(
                            s_ps[:],
                            lhsT=qT_sb[:, col:col + 1],
                            rhs=kt_sb[:, col * S + lo: col * S + lo + 512],
                            start=True,
                            stop=True,
                        )
                        sr = srpool.tile([1, 512], f32r, tag="sr")
                        if h % 2 == 0:
                            nc.vector.tensor_copy(sr[:], s_ps[:])
                        else:
                            nc.scalar.copy(sr[:], s_ps[:])
                        nc.tensor.matmul(
                            sp[:],
                            lhsT=basis_sb[:, h * HPC:(h + 1) * HPC],
                            rhs=sr[:],
                            start=(h == 0),
                            stop=(h == HPC - 1),
                        )
                sf = sfpool.tile([HPC, 512], f32, tag="sf")
                if sl[b] >= lo + 512:
                    nc.vector.tensor_add(
                        sf[:], sp[:], term1_sb[:, lo: lo + 512]
                    )
                elif sl[b] > lo:
                    w = sl[b] - lo
                    nc.vector.tensor_add(
                        sf[:, :w], sp[:, :w], term1_sb[:, lo: lo + w]
                    )
                    nc.vector.memset(sf[:, w:], NEG)
                else:
                    nc.vector.memset(sf[:], NEG)
                nc.scalar.activation(
                    attn_sb[:, b * S + lo: b * S + lo + 512],
                    sf[:],
                    func=mybir.ActivationFunctionType.Exp,
                    bias=term2_sb[:, b:b + 1],
                    accum_out=sums_sb[b][:, nt:nt + 1],
                )

        # ---- transpose attn chunks: [5, 128] -> [128, 5] ----
        for c in range(16):
            for b in range(B):
                ta = psum.tile([128, HPC], bf16, tag="ps", name=f"ta{c}_{b}")
                nc.tensor.transpose(
                    ta[:],
                    attn_sb[:, b * S + c * 128: b * S + (c + 1) * 128],
                    ident_sb[:HPC, :HPC],
                )
                nc.vector.tensor_copy(
                    attnT_sb[:, c * 20 + b * HPC: c * 20 + (b + 1) * HPC], ta[:]
                )

        # ---- softmax denominators (per sequence, partitions 0..4) ----
        for b in range(B):
            nc.vector.tensor_add(
                sum2_sb[b][:, 0:1], sums_sb[b][:, 0:1], sums_sb[b][:, 1:2]
            )
            nc.vector.tensor_add(
                sum2_sb[b][:, 1:2], sums_sb[b][:, 2:3], sums_sb[b][:, 3:4]
            )
            nc.vector.tensor_add(
                sumt_sb[b][:], sum2_sb[b][:, 0:1], sum2_sb[b][:, 1:2]
            )
            nc.vector.reciprocal(recip_sb[b][:], sumt_sb[b][:])

        # ---- attn @ V, batched over the 5 heads of each sequence ----
        for b in range(B):
            if b >= 2:
                vt_tiles[b] = vpool.tile([128, 16, EPC], bf16, tag="V", name=f"vt{b}")
                nc.sync.dma_start(out=vt_tiles[b][:], in_=vt[b])
                nc.gpsimd.dma_start(
                    out=vt_tiles[b][pos[b] % 128: pos[b] % 128 + 1, pos[b] // 128, :],
                    in_=qkv_sb[b:b + 1, 2 * EPC:3 * EPC],
                )
            ao0 = psum.tile([HPC, 512], f32, tag="ps", name=f"ao0_{b}")
            ao1 = psum.tile([HPC, EPC - 512], f32, tag="ps", name=f"ao1_{b}")
            for c in range(16):
                lt = attnT_sb[:, c * 20 + b * HPC: c * 20 + (b + 1) * HPC]
                nc.tensor.matmul(
                    ao0[:], lhsT=lt, rhs=vt_tiles[b][:, c, 0:512],
                    start=(c == 0), stop=(c == 15),
                )
                nc.tensor.matmul(
                    ao1[:], lhsT=lt, rhs=vt_tiles[b][:, c, 512:EPC],
                    start=(c == 0), stop=(c == 15),
                )
            nc.scalar.activation(
                ao_sb[b][:, 0:512], ao0[:],
                func=mybir.ActivationFunctionType.Copy, scale=recip_sb[b][:],
            )
            nc.scalar.activation(
                ao_sb[b][:, 512:EPC], ao1[:],
                func=mybir.ActivationFunctionType.Copy, scale=recip_sb[b][:],
            )

        # ---- transpose attn_out diag blocks -> aoT [128, 20] (col h*B+b) ----
        for b in range(B):
            for h in range(HPC):
                to = psum.tile([128, HPC], bf16, tag="ps", name=f"to{b}_{h}")
                nc.tensor.transpose(
                    to[:], ao_sb[b][:, h * 128:(h + 1) * 128], ident_sb[:HPC, :HPC]
                )
                nc.vector.tensor_copy(
                    aoT_sb[:, h * B + b: h * B + b + 1], to[:, h:h + 1]
                )

        # ---- output projection: out[4, 5120] ----
        for jg in range(10):
            wt = wopool.tile([128, HPC * 512], bf16, tag="wo", name=f"wo{jg}")
            nc.sync.dma_start(out=wt[:], in_=wo[:, :, jg * 512:(jg + 1) * 512])
            op = psum.tile([B, 512], f32, tag="ps", name=f"op{jg}")
            for hc in range(HPC):
                nc.tensor.matmul(
                    op[:],
                    lhsT=aoT_sb[:, hc * B:(hc + 1) * B],
                    rhs=wt[:, hc * 512:(hc + 1) * 512],
                    start=(hc == 0),
                    stop=(hc == HPC - 1),
                )
            nc.scalar.copy(out_sb[:, jg * 512:(jg + 1) * 512], op[:])

        nc.sync.dma_start(out=outp[:], in_=out_sb[:])

    nc.compile()
    return nc


def _bf16(x):
    return np.ascontiguousarray(x.astype(BF16))


def _prepare_core_inputs(core, hT_full, qkv_bf, o_bf, k_bf, v_bf, bt, sl, pos,
                         ident, basis, slopes_all):
    hs = slice(core * HPC, (core + 1) * HPC)
    es = slice(core * EPC, (core + 1) * EPC)

    # Wcat [128, 40, 1920]: Wcat[p, kc, j] = W[kc*128+p, j]; q pre-scaled.
    wcat = np.concatenate(
        [qkv_bf[0][:, es], qkv_bf[1][:, es], qkv_bf[2][:, es]], axis=1
    )  # [5120, 1920] bf16
    wcat = np.ascontiguousarray(wcat.reshape(NKC, 128, NQKV).transpose(1, 0, 2))

    # kt [128(d), (h*B+b)*S + t]
    kg = k_bf[:, hs]   # [NB, 5, 16, 128] bf16
    kt = np.empty((128, HPC, B, S), BF16)
    for b in range(B):
        kk = kg[bt[b]]                              # [128blk, 5, 16, 128]
        kk = kk.transpose(1, 0, 2, 3).reshape(HPC, S, D)
        kt[:, :, b, :] = kk.transpose(2, 0, 1)      # [d, h, t]
    kt = kt.reshape(128, HPC * B * S)

    # vt [B, 128(t%128), 16(t//128), 640(h*128+d)]
    vg = v_bf[:, hs]
    vtb = np.empty((B, 128, 16, EPC), BF16)
    for b in range(B):
        vv = vg[bt[b]]                              # [128blk, 5, 16, 128]
        vv = vv.transpose(0, 2, 1, 3).reshape(S, HPC, D)   # [t, h, d]
        vtb[b] = vv.reshape(16, 128, HPC * D).transpose(1, 0, 2)

    # wo [128, 5, 5120]: wo[p, h, j] = Wo[h*128+p, j]
    wo = np.ascontiguousarray(o_bf[es, :].reshape(HPC, 128, E).transpose(1, 0, 2))

    # alibi decomposition
    slopes = slopes_all[core * HPC:(core + 1) * HPC]
    t_idx = np.arange(S, dtype=np.float32)
    term1 = np.ascontiguousarray(slopes[:, None] * t_idx[None, :])
    term2 = np.ascontiguousarray(
        -slopes[:, None] * np.asarray(pos, np.float32)[None, :]
    )

    return dict(hT=hT_full, wcat=wcat, kt=kt, vt=vtb, wo=wo,
                term1=term1, term2=term2, basis=basis, ident=ident)


def kernel(**inputs):
    global LAST_RESULTS
    hidden = np.asarray(inputs["hidden_states"], np.float32)
    qkv_w = np.asarray(inputs["qkv_weight"], np.float32)
    o_w = np.asarray(inputs["o_proj_weight"], np.float32)
    k_cache = np.asarray(inputs["k_cache"], np.float32)
    v_cache = np.asarray(inputs["v_cache"], np.float32)
    bt = np.asarray(inputs["block_tables"]).astype(np.int64)
    sl = np.asarray(inputs["sequence_lengths"]).astype(np.int64)

    pos = tuple(int(x) - 1 for x in sl)

    # Shared host-side conversions (bf16 once, slice per core after).
    qkv_bf = [
        _bf16(qkv_w[0] * np.float32(D ** -0.5)),
        _bf16(qkv_w[1]),
        _bf16(qkv_w[2]),
    ]
    o_bf = _bf16(o_w)
    k_bf = _bf16(k_cache)
    v_bf = _bf16(v_cache)
    hT_full = _bf16(
        hidden.T.reshape(NKC, 128, B).transpose(1, 0, 2).reshape(128, NKC * B)
    )
    ident = np.eye(20, dtype=BF16)
    basis = np.ascontiguousarray(
        np.eye(HPC, dtype=np.float32).reshape(1, HPC * HPC)
    )
    slopes_all = _alibi_slopes(H)

    in_maps = [
        _prepare_core_inputs(c, hT_full, qkv_bf, o_bf, k_bf, v_bf, bt, sl, pos,
                             ident, basis, slopes_all)
        for c in range(NCORES)
    ]

    if pos not in _PROGRAM_CACHE:
        _PROGRAM_CACHE[pos] = _build_program(pos)
    nc = _PROGRAM_CACHE[pos]

    from concourse.bass_utils import run_bass_kernel_spmd

    res = run_bass_kernel_spmd(
        nc,
        in_maps,
        core_ids=list(range(NCORES)),
        trace=bool(os.environ.get("BASS_TRACE")),
    )
    LAST_RESULTS = res

    out = np.zeros((B, E), np.float64)
    for c in range(NCORES):
        out += np.asarray(res.results[c]["outp"]).astype(np.float64)
    return out.astype(np.float32)


# revision 14
# speedup vs baseline: 2.9668x; 1.5206x over previous
"""Paged KV-cache decode attention with ALiBi (Baichuan-style), fused
QKV + attention + output projection, tensor-parallel over heads across
8 Trainium2 NeuronCores.

v3 design (bf16, long-moving-dim matmuls, masked-stationary scores):
  - All matmul operands bf16 (1 cycle/moving-row vs 4 for fp32); PSUM
    accumulation stays fp32. Halves DMA bytes vs fp32.
  - Matmuls are oriented so the STATIONARY operand is tiny (4-20 cols)
    and the MOVING operand streams ~512 columns per instruction:
      * QKV:   out[4,1920] = x @ Wcat, stationary = xT chunk [128,4]
      * scores: PE psum writes must start at a quadrant base, so row
        r = b*5+h of the shared [20,512] psum tile is produced by a
        MASKED stationary [128,20] that is zero except column r
        (garbage rows accumulate +0). 20 accumulating matmuls fill
        the tile; softmax then runs on all 20 partitions at once.
      * attn@V: batched over the 5 heads of one sequence via a
        [128(t),5] stationary of transposed probs against a
        [128(t), 5*128(h,d)] V chunk (block-diagonal extraction)
      * o_proj: out[4,5120], stationary = attn_out^T chunk [128,4]
  - ALiBi bias + sequence mask baked into a host [20,2048] tensor
    (slope_h*t, -1e30 past seq_len); the -slope_h*pos_b term is a
    per-partition scalar folded into the Exp activation bias.
  - Small transposes (q/k/attn/attn_out/recip) on the PE with an
    identity stationary; softmax row sums via activation accum_out;
    normalization folded into the psum->sbuf copy (per-partition
    scale, per-sequence [5,1] tiles).
  - K cache staged host-side as one [128(d), 20*2048] bf16 image per
    core (single 10.5MB DMA); V as [128(t%128), 16, 640] per sequence.
"""

import math
import os
import sys
from contextlib import ExitStack

import numpy as np
import ml_dtypes

sys.path.insert(0, "/opt/trn_rl_repo")

BF16 = ml_dtypes.bfloat16

B = 4
E = 5120
H = 40
D = 128
BS = 16
NB = 512
MB = 128
S = MB * BS  # 2048
NCORES = 8
HPC = H // NCORES   # 5 heads per core
EPC = HPC * D       # 640
NKC = E // 128      # 40 contraction chunks
NQKV = 3 * EPC      # 1920 qkv output columns per core
R = HPC * B         # 20 (b,h) pairs per core
NEG = -1.0e30


def _alibi_slopes(num_heads):
    cp2 = 2 ** int(math.floor(math.log2(num_heads)))
    base = 2.0 ** (-(2.0 ** (-(math.log2(cp2) - 3))))
    slopes = base ** np.arange(1, cp2 + 1, dtype=np.float64)
    if cp2 != num_heads:
        extra_base = 2.0 ** (-(2.0 ** (-(math.log2(2 * cp2) - 3))))
        n_rem = min(cp2, num_heads - cp2)
        extra = extra_base ** np.arange(1, 1 + 2 * n_rem, 2, dtype=np.float64)
        slopes = np.concatenate([slopes, extra])
    return slopes.astype(np.float32)


_PROGRAM_CACHE = {}
LAST_RESULTS = None  # BassKernelResults of the most recent run (for test.py)


def _build_program(pos):
    import concourse.bacc as bacc
    import concourse.bass as bass
    import concourse.tile as tile
    from concourse import mybir

    f32 = mybir.dt.float32
    bf16 = mybir.dt.bfloat16
    nc = bacc.Bacc()
    sl = tuple(p + 1 for p in pos)

    hT = nc.declare_dram_parameter("hT", [128, NKC * B], bf16, isOutput=False)
    wcat = nc.declare_dram_parameter("wcat", [128, NKC, NQKV], bf16, isOutput=False)
    kt = nc.declare_dram_parameter("kt", [128, R * S], bf16, isOutput=False)
    vt = nc.declare_dram_parameter("vt", [B, 128, 16, EPC], bf16, isOutput=False)
    wo = nc.declare_dram_parameter("wo", [128, HPC, E], bf16, isOutput=False)
    term1 = nc.declare_dram_parameter("term1", [R, S], f32, isOutput=False)
    term2 = nc.declare_dram_parameter("term2", [R, 1], f32, isOutput=False)
    ident = nc.declare_dram_parameter("ident", [20, 20], bf16, isOutput=False)
    identf = nc.declare_dram_parameter("identf", [20, 20], f32, isOutput=False)
    outp = nc.declare_dram_parameter("outp", [B, E], bf16, isOutput=True)

    with tile.TileContext(nc) as tc, ExitStack() as ctx:
        consts = ctx.enter_context(tc.tile_pool(name="consts", bufs=1))
        ktpool = ctx.enter_context(tc.tile_pool(name="ktpool", bufs=1))
        wpool = ctx.enter_context(tc.tile_pool(name="wpool", bufs=3))
        vpool = ctx.enter_context(tc.tile_pool(name="vpool", bufs=2))
        wopool = ctx.enter_context(tc.tile_pool(name="wopool", bufs=4))
        sfpool = ctx.enter_context(tc.tile_pool(name="sfpool", bufs=2))
        psum = ctx.enter_context(tc.tile_pool(name="psum", bufs=8, space="PSUM"))

        # ---- persistent tiles; scalar ring carries wcat then kt ----
        hT_sb = consts.tile([128, NKC * B], bf16)
        nc.scalar.dma_start(out=hT_sb[:], in_=hT[:])
        ident_sb = consts.tile([20, 20], bf16)
        nc.scalar.dma_start(out=ident_sb[:], in_=ident[:])
        identf_sb = consts.tile([20, 20], f32)
        nc.scalar.dma_start(out=identf_sb[:], in_=identf[:])
        term1_sb = consts.tile([R, S], f32)
        nc.scalar.dma_start(out=term1_sb[:], in_=term1[:])
        term2_sb = consts.tile([R, 1], f32)
        nc.scalar.dma_start(out=term2_sb[:], in_=term2[:])

        vt_tiles = [None] * B
        for b in range(2):
            vt_tiles[b] = vpool.tile([128, 16, EPC], bf16, tag="V", name=f"vt{b}")
            nc.sync.dma_start(out=vt_tiles[b][:], in_=vt[b])

        qkv_sb = consts.tile([B, NQKV], bf16)
        qTm_sb = consts.tile([128, 21 * R + 1], bf16)  # masked: col r*21 live
        kT_sb = consts.tile([128, R], bf16)         # col = h*B + b
        attn_sb = consts.tile([R, S], bf16)         # row r = b*5+h
        attnT_sb = consts.tile([128, 16 * R], bf16)  # col = c*20 + b*5 + h
        sums_sb = consts.tile([R, 4], f32)
        sum2_sb = consts.tile([R, 2], f32)
        sumt_sb = consts.tile([R, 1], f32)
        recip_sb = consts.tile([R, 1], f32)
        recip_row = consts.tile([1, R], f32)
        recip_b = [consts.tile([HPC, 1], f32, name=f"recipb{b}") for b in range(B)]
        ao_sb = [consts.tile([HPC, EPC], bf16, name=f"ao{b}") for b in range(B)]
        aoT_sb = consts.tile([128, R], bf16)        # col = h*B + b
        out_sb = consts.tile([B, E], bf16)

        nc.vector.memset(qTm_sb[:], 0.0)

        # ---- fused QKV projection: qkv[4, 1920] ----
        qkv_ps = [
            psum.tile([B, min(512, NQKV - nt * 512)], f32, tag="ps", name=f"qkv_ps{nt}")
            for nt in range(4)
        ]
        for g in range(NKC // 2):
            wt = wpool.tile([128, 2 * NQKV], bf16, tag="w")
            nc.scalar.dma_start(out=wt[:], in_=wcat[:, 2 * g:2 * g + 2, :])
            for kl in range(2):
                kc = 2 * g + kl
                for nt in range(4):
                    w = min(512, NQKV - nt * 512)
                    nc.tensor.matmul(
                        qkv_ps[nt][:],
                        lhsT=hT_sb[:, kc * B:(kc + 1) * B],
                        rhs=wt[:, kl * NQKV + nt * 512: kl * NQKV + nt * 512 + w],
                        start=(kc == 0),
                        stop=(kc == NKC - 1),
                    )
        # K cache image after the weights on the same ring (needed ~90us in)
        kt_sb = ktpool.tile([128, R * S], bf16)
        nc.scalar.dma_start(out=kt_sb[:], in_=kt[:])

        for nt in range(4):
            w = min(512, NQKV - nt * 512)
            nc.scalar.copy(qkv_sb[:, nt * 512: nt * 512 + w], qkv_ps[nt][:])

        # ---- transpose q into masked stationaries, k into kT ----
        # q head h -> columns {21*(b*5+h) : b} of qTm (stride 105)
        for h in range(HPC):
            tq = psum.tile([128, B], bf16, tag="ps", name=f"tq_{h}")
            nc.tensor.transpose(
                tq[:], qkv_sb[:, h * 128:(h + 1) * 128], ident_sb[:B, :B]
            )
            dst = qTm_sb[:, :420].rearrange("p (b rest) -> p b rest", b=B, rest=105)
            nc.vector.tensor_copy(dst[:, :, 21 * h], tq[:])
        for h in range(HPC):
            tk = psum.tile([128, B], bf16, tag="ps", name=f"tk_{h}")
            nc.tensor.transpose(
                tk[:], qkv_sb[:, EPC + h * 128: EPC + (h + 1) * 128],
                ident_sb[:B, :B],
            )
            nc.vector.tensor_copy(kT_sb[:, h * B:(h + 1) * B], tk[:])

        # ---- scatter new-token K column / V row ----
        for h in range(HPC):
            for b in range(B):
                col = h * B + b
                nc.vector.tensor_copy(
                    kt_sb[:, col * S + pos[b]: col * S + pos[b] + 1],
                    kT_sb[:, col:col + 1],
                )
        for b in range(2):
            nc.gpsimd.dma_start(
                out=vt_tiles[b][pos[b] % 128: pos[b] % 128 + 1, pos[b] // 128, :],
                in_=qkv_sb[b:b + 1, 2 * EPC:3 * EPC],
            )

        # ---- scores + softmax on [20, 512] tiles ----
        for nt in range(4):
            lo = nt * 512
            sp = psum.tile([R, 512], f32, tag="ps", name=f"sp{nt}")
            live = [r for r in range(R) if sl[r // HPC] > lo]
            for i, r in enumerate(live):
                b, h = divmod(r, HPC)
                col = h * B + b
                nc.tensor.matmul(
                    sp[:],
                    lhsT=qTm_sb[:, r * R:(r + 1) * R],
                    rhs=kt_sb[:, col * S + lo: col * S + lo + 512],
                    start=(i == 0),
                    stop=(i == len(live) - 1),
                )
            sf = sfpool.tile([R, 512], f32, tag="sf")
            nc.vector.tensor_add(sf[:], sp[:], term1_sb[:, lo: lo + 512])
            nc.scalar.activation(
                attn_sb[:, lo: lo + 512],
                sf[:],
                func=mybir.ActivationFunctionType.Exp,
                bias=term2_sb[:],
                accum_out=sums_sb[:, nt:nt + 1],
            )

        # ---- transpose attn chunks: [20, 128] -> [128, 20] ----
        for c in range(16):
            ta = psum.tile([128, R], bf16, tag="ps", name=f"ta{c}")
            nc.tensor.transpose(ta[:], attn_sb[:, c * 128:(c + 1) * 128], ident_sb[:])
            nc.vector.tensor_copy(attnT_sb[:, c * R:(c + 1) * R], ta[:])

        # ---- softmax denominators -> per-sequence [5,1] recip tiles ----
        nc.vector.tensor_add(sum2_sb[:, 0:1], sums_sb[:, 0:1], sums_sb[:, 1:2])
        nc.vector.tensor_add(sum2_sb[:, 1:2], sums_sb[:, 2:3], sums_sb[:, 3:4])
        nc.vector.tensor_add(sumt_sb[:], sum2_sb[:, 0:1], sum2_sb[:, 1:2])
        nc.vector.reciprocal(recip_sb[:], sumt_sb[:])
        rr = psum.tile([1, R], f32, tag="ps", name="rr")
        nc.tensor.transpose(rr[:], recip_sb[:], identf_sb[:])
        nc.vector.tensor_copy(recip_row[:], rr[:])
        for b in range(B):
            rb = psum.tile([HPC, 1], f32, tag="ps", name=f"rb{b}")
            nc.tensor.transpose(
                rb[:], recip_row[:, b * HPC:(b + 1) * HPC], identf_sb[:1, :1]
            )
            nc.vector.tensor_copy(recip_b[b][:], rb[:])

        # ---- attn @ V, batched over the 5 heads of each sequence ----
        for b in range(B):
            if b >= 2:
                vt_tiles[b] = vpool.tile([128, 16, EPC], bf16, tag="V", name=f"vt{b}")
                nc.sync.dma_start(out=vt_tiles[b][:], in_=vt[b])
                nc.gpsimd.dma_start(
                    out=vt_tiles[b][pos[b] % 128: pos[b] % 128 + 1, pos[b] // 128, :],
                    in_=qkv_sb[b:b + 1, 2 * EPC:3 * EPC],
                )
            ao0 = psum.tile([HPC, 512], f32, tag="ps", name=f"ao0_{b}")
            ao1 = psum.tile([HPC, EPC - 512], f32, tag="ps", name=f"ao1_{b}")
            nch = (sl[b] + 127) // 128
            for c in range(nch):
                lt = attnT_sb[:, c * R + b * HPC: c * R + (b + 1) * HPC]
                nc.tensor.matmul(
                    ao0[:], lhsT=lt, rhs=vt_tiles[b][:, c, 0:512],
                    start=(c == 0), stop=(c == nch - 1),
                )
                nc.tensor.matmul(
                    ao1[:], lhsT=lt, rhs=vt_tiles[b][:, c, 512:EPC],
                    start=(c == 0), stop=(c == nch - 1),
                )
            nc.scalar.activation(
                ao_sb[b][:, 0:512], ao0[:],
                func=mybir.ActivationFunctionType.Copy, scale=recip_b[b][:],
            )
            nc.scalar.activation(
                ao_sb[b][:, 512:EPC], ao1[:],
                func=mybir.ActivationFunctionType.Copy, scale=recip_b[b][:],
            )

        # ---- transpose attn_out diag blocks -> aoT [128, 20] (col h*B+b) ----
        for b in range(B):
            for h in range(HPC):
                to = psum.tile([128, HPC], bf16, tag="ps", name=f"to{b}_{h}")
                nc.tensor.transpose(
                    to[:], ao_sb[b][:, h * 128:(h + 1) * 128], ident_sb[:HPC, :HPC]
                )
                nc.vector.tensor_copy(
                    aoT_sb[:, h * B + b: h * B + b + 1], to[:, h:h + 1]
                )

        # ---- output projection: out[4, 5120] ----
        for jg in range(10):
            wt = wopool.tile([128, HPC * 512], bf16, tag="wo", name=f"wo{jg}")
            nc.sync.dma_start(out=wt[:], in_=wo[:, :, jg * 512:(jg + 1) * 512])
            op = psum.tile([B, 512], f32, tag="ps", name=f"op{jg}")
            for hc in range(HPC):
                nc.tensor.matmul(
                    op[:],
                    lhsT=aoT_sb[:, hc * B:(hc + 1) * B],
                    rhs=wt[:, hc * 512:(hc + 1) * 512],
                    start=(hc == 0),
                    stop=(hc == HPC - 1),
                )
            nc.scalar.copy(out_sb[:, jg * 512:(jg + 1) * 512], op[:])

        nc.sync.dma_start(out=outp[:], in_=out_sb[:])

    nc.compile()
    return nc


def _bf16(x):
    return np.ascontiguousarray(x.astype(BF16))


def _prepare_core_inputs(core, hT_full, qkv_bf, o_bf, k_bf, v_bf, bt, sl, pos,
                         ident, identf, slopes_all):
    hs = slice(core * HPC, (core + 1) * HPC)
    es = slice(core * EPC, (core + 1) * EPC)

    # Wcat [128, 40, 1920]: Wcat[p, kc, j] = W[kc*128+p, j]; q pre-scaled.
    wcat = np.concatenate(
        [qkv_bf[0][:, es], qkv_bf[1][:, es], qkv_bf[2][:, es]], axis=1
    )  # [5120, 1920] bf16
    wcat = np.ascontiguousarray(wcat.reshape(NKC, 128, NQKV).transpose(1, 0, 2))

    # kt [128(d), (h*B+b)*S + t]
    kg = k_bf[:, hs]   # [NB, 5, 16, 128] bf16
    kt = np.empty((128, HPC, B, S), BF16)
    for b in range(B):
        kk = kg[bt[b]]                              # [128blk, 5, 16, 128]
        kk = kk.transpose(1, 0, 2, 3).reshape(HPC, S, D)
        kt[:, :, b, :] = kk.transpose(2, 0, 1)      # [d, h, t]
    kt = kt.reshape(128, HPC * B * S)

    # vt [B, 128(t%128), 16(t//128), 640(h*128+d)]
    vg = v_bf[:, hs]
    vtb = np.empty((B, 128, 16, EPC), BF16)
    for b in range(B):
        vv = vg[bt[b]]                              # [128blk, 5, 16, 128]
        vv = vv.transpose(0, 2, 1, 3).reshape(S, HPC, D)   # [t, h, d]
        vtb[b] = vv.reshape(16, 128, HPC * D).transpose(1, 0, 2)

    # wo [128, 5, 5120]: wo[p, h, j] = Wo[h*128+p, j]
    wo = np.ascontiguousarray(o_bf[es, :].reshape(HPC, 128, E).transpose(1, 0, 2))

    # alibi: term1[r, t] = slope_h * t (masked), term2[r] = -slope_h*pos_b
    slopes = slopes_all[core * HPC:(core + 1) * HPC]
    t_idx = np.arange(S, dtype=np.float32)
    term1 = np.empty((B, HPC, S), np.float32)
    term2 = np.empty((B, HPC, 1), np.float32)
    for b in range(B):
        term1[b] = slopes[:, None] * t_idx[None, :]
        term1[b, :, sl[b]:] = NEG
        term2[b, :, 0] = -slopes * np.float32(pos[b])

    return dict(hT=hT_full, wcat=wcat, kt=kt, vt=vtb, wo=wo,
                term1=np.ascontiguousarray(term1.reshape(R, S)),
                term2=np.ascontiguousarray(term2.reshape(R, 1)),
                ident=ident, identf=identf)


def kernel(**inputs):
    global LAST_RESULTS
    hidden = np.asarray(inputs["hidden_states"], np.float32)
    qkv_w = np.asarray(inputs["qkv_weight"], np.float32)
    o_w = np.asarray(inputs["o_proj_weight"], np.float32)
    k_cache = np.asarray(inputs["k_cache"], np.float32)
    v_cache = np.asarray(inputs["v_cache"], np.float32)
    bt = np.asarray(inputs["block_tables"]).astype(np.int64)
    sl = np.asarray(inputs["sequence_lengths"]).astype(np.int64)

    pos = tuple(int(x) - 1 for x in sl)

    # Shared host-side conversions (bf16 once, slice per core after).
    qkv_bf = [
        _bf16(qkv_w[0] * np.float32(D ** -0.5)),
        _bf16(qkv_w[1]),
        _bf16(qkv_w[2]),
    ]
    o_bf = _bf16(o_w)
    k_bf = _bf16(k_cache)
    v_bf = _bf16(v_cache)
    hT_full = _bf16(
        hidden.T.reshape(NKC, 128, B).transpose(1, 0, 2).reshape(128, NKC * B)
    )
    ident = np.eye(20, dtype=BF16)
    identf = np.eye(20, dtype=np.float32)
    slopes_all = _alibi_slopes(H)

    in_maps = [
        _prepare_core_inputs(c, hT_full, qkv_bf, o_bf, k_bf, v_bf, bt, sl, pos,
                             ident, identf, slopes_all)
        for c in range(NCORES)
    ]

    if pos not in _PROGRAM_CACHE:
        _PROGRAM_CACHE[pos] = _build_program(pos)
    nc = _PROGRAM_CACHE[pos]

    from concourse.bass_utils import run_bass_kernel_spmd

    res = run_bass_kernel_spmd(
        nc,
        in_maps,
        core_ids=list(range(NCORES)),
        trace=bool(os.environ.get("BASS_TRACE")),
    )
    LAST_RESULTS = res

    out = np.zeros((B, E), np.float64)
    for c in range(NCORES):
        out += np.asarray(res.results[c]["outp"]).astype(np.float64)
    return out.astype(np.float32)


# revision 15
# speedup vs baseline: 3.2576x; 1.0980x over previous
"""Paged KV-cache decode attention with ALiBi (Baichuan-style), fused
QKV + attention + output projection, tensor-parallel over heads across
8 Trainium2 NeuronCores.

v4 design (bf16, long-moving-dim matmuls, masked-stationary scores,
sequence-length-truncated K/V, ordered DMA rings):
  - All matmul operands bf16; PSUM accumulation fp32.
  - QKV: out[4,1920], stationary = xT chunk [128,4], moving = weight
    columns (512/psum bank); weights stream through SBUF double-buffered.
  - Scores row r = b*5+h of a shared [20,512] psum tile produced by a
    MASKED stationary [128,20] (zero except column r) so the PE can
    write every row despite the quadrant base restriction; 20
    accumulating matmuls per tile, then 20-lane bias add + Exp.
  - K cache: packed per nt-section, only rows with sl_b > nt*512, in
    one [128, n_live*512] bf16 image; 4 section DMAs let scores of
    section nt start as soon as its bytes land.
  - V cache: per-sequence [128(t%128), nch_b, 640(h,d)] truncated
    tiles, all resident; attn@V batches the 5 heads of a sequence via
    a [128,5] stationary of transposed probs (block-diag extraction).
  - ALiBi bias + sequence mask baked into a host [20,2048] tensor
    (slope_h*t, -1e30 past seq_len); -slope_h*pos_b enters as the Exp
    activation's per-partition bias.
  - softmax row sums via activation accum_out; normalization by per-
    sequence [5,1] reciprocal tiles folded into the psum->sbuf copy.
  - DMA: scalar (ACT) HWDGE ring carries smalls -> wcat pairs -> kt
    sections -> vt tiles in consumption order; sync (SP) ring carries
    wo slices + output store; gpsimd handles the two-line scatters.
"""

import math
import os
import sys
from contextlib import ExitStack

import numpy as np
import ml_dtypes

sys.path.insert(0, "/opt/trn_rl_repo")

BF16 = ml_dtypes.bfloat16

B = 4
E = 5120
H = 40
D = 128
BS = 16
NB = 512
MB = 128
S = MB * BS  # 2048
NCORES = 8
HPC = H // NCORES   # 5 heads per core
EPC = HPC * D       # 640
NKC = E // 128      # 40 contraction chunks
NQKV = 3 * EPC      # 1920 qkv output columns per core
R = HPC * B         # 20 (b,h) pairs per core
NEG = -1.0e30


def _alibi_slopes(num_heads):
    cp2 = 2 ** int(math.floor(math.log2(num_heads)))
    base = 2.0 ** (-(2.0 ** (-(math.log2(cp2) - 3))))
    slopes = base ** np.arange(1, cp2 + 1, dtype=np.float64)
    if cp2 != num_heads:
        extra_base = 2.0 ** (-(2.0 ** (-(math.log2(2 * cp2) - 3))))
        n_rem = min(cp2, num_heads - cp2)
        extra = extra_base ** np.arange(1, 1 + 2 * n_rem, 2, dtype=np.float64)
        slopes = np.concatenate([slopes, extra])
    return slopes.astype(np.float32)


def _kt_sections(sl):
    """live rows (r-order) per nt section and their packed offsets."""
    live = [[r for r in range(R) if sl[r // HPC] > nt * 512] for nt in range(4)]
    off = [0] * 4
    acc = 0
    for nt in range(4):
        off[nt] = acc
        acc += len(live[nt])
    return live, off, acc


_PROGRAM_CACHE = {}
LAST_RESULTS = None  # BassKernelResults of the most recent run (for test.py)


def _build_program(pos):
    import concourse.bacc as bacc
    import concourse.bass as bass
    import concourse.tile as tile
    from concourse import mybir

    f32 = mybir.dt.float32
    bf16 = mybir.dt.bfloat16
    nc = bacc.Bacc()
    sl = tuple(p + 1 for p in pos)
    nch = [(s + 127) // 128 for s in sl]
    live, off, nlive = _kt_sections(sl)

    hT = nc.declare_dram_parameter("hT", [128, NKC * B], bf16, isOutput=False)
    wcat = nc.declare_dram_parameter("wcat", [128, NKC, NQKV], bf16, isOutput=False)
    kt = nc.declare_dram_parameter("kt", [128, nlive * 512], bf16, isOutput=False)
    vt = [
        nc.declare_dram_parameter(f"vt{b}", [128, nch[b], EPC], bf16, isOutput=False)
        for b in range(B)
    ]
    wo = nc.declare_dram_parameter("wo", [128, HPC, E], bf16, isOutput=False)
    term1 = nc.declare_dram_parameter("term1", [R, S], f32, isOutput=False)
    term2 = nc.declare_dram_parameter("term2", [R, 1], f32, isOutput=False)
    ident = nc.declare_dram_parameter("ident", [20, 20], bf16, isOutput=False)
    identf = nc.declare_dram_parameter("identf", [20, 20], f32, isOutput=False)
    outp = nc.declare_dram_parameter("outp", [B, E], bf16, isOutput=True)

    with tile.TileContext(nc) as tc, ExitStack() as ctx:
        consts = ctx.enter_context(tc.tile_pool(name="consts", bufs=1))
        wpool = ctx.enter_context(tc.tile_pool(name="wpool", bufs=3))
        wopool = ctx.enter_context(tc.tile_pool(name="wopool", bufs=2))
        sfpool = ctx.enter_context(tc.tile_pool(name="sfpool", bufs=2))
        psum = ctx.enter_context(tc.tile_pool(name="psum", bufs=8, space="PSUM"))

        # ---- small constants first on the scalar (ACT) ring ----
        hT_sb = consts.tile([128, NKC * B], bf16)
        nc.scalar.dma_start(out=hT_sb[:], in_=hT[:])
        ident_sb = consts.tile([20, 20], bf16)
        nc.scalar.dma_start(out=ident_sb[:], in_=ident[:])
        identf_sb = consts.tile([20, 20], f32)
        nc.scalar.dma_start(out=identf_sb[:], in_=identf[:])
        term1_sb = consts.tile([R, S], f32)
        nc.scalar.dma_start(out=term1_sb[:], in_=term1[:])
        term2_sb = consts.tile([R, 1], f32)
        nc.scalar.dma_start(out=term2_sb[:], in_=term2[:])

        qkv_sb = consts.tile([B, NQKV], bf16)
        qTm_sb = consts.tile([128, 21 * R + 1], bf16)  # masked: col r*21 live
        kT_sb = consts.tile([128, R], bf16)            # col = r = b*5+h
        attn_sb = consts.tile([R, S], bf16)            # row r
        attnT_sb = consts.tile([128, 16 * R], bf16)    # col = c*20 + r
        sums_sb = consts.tile([R, 4], f32)
        sum2_sb = consts.tile([R, 2], f32)
        sumt_sb = consts.tile([R, 1], f32)
        recip_sb = consts.tile([R, 1], f32)
        recip_row = consts.tile([1, R], f32)
        recip_b = [consts.tile([HPC, 1], f32, name=f"recipb{b}") for b in range(B)]
        ao_sb = [consts.tile([HPC, EPC], bf16, name=f"ao{b}") for b in range(B)]
        aoT_sb = consts.tile([128, R], bf16)           # col = h*B + b
        out_sb = consts.tile([B, E], bf16)

        nc.vector.memset(qTm_sb[:], 0.0)

        # ---- fused QKV projection: qkv[4, 1920] (wcat pairs stream) ----
        qkv_ps = [
            psum.tile([B, min(512, NQKV - nt * 512)], f32, tag="ps", name=f"qkv_ps{nt}")
            for nt in range(4)
        ]
        for g in range(NKC // 2):
            wt = wpool.tile([128, 2 * NQKV], bf16, tag="w")
            nc.scalar.dma_start(out=wt[:], in_=wcat[:, 2 * g:2 * g + 2, :])
            for kl in range(2):
                kc = 2 * g + kl
                for nt in range(4):
                    w = min(512, NQKV - nt * 512)
                    nc.tensor.matmul(
                        qkv_ps[nt][:],
                        lhsT=hT_sb[:, kc * B:(kc + 1) * B],
                        rhs=wt[:, kl * NQKV + nt * 512: kl * NQKV + nt * 512 + w],
                        start=(kc == 0),
                        stop=(kc == NKC - 1),
                    )

        # K sections then V tiles, in consumption order, same ring
        kt_sb = consts.tile([128, nlive * 512], bf16)
        for nt in range(4):
            lo, hi = off[nt] * 512, (off[nt] + len(live[nt])) * 512
            nc.scalar.dma_start(out=kt_sb[:, lo:hi], in_=kt[:, lo:hi])
        vt_sb = []
        for b in range(B):
            vtile = consts.tile([128, nch[b], EPC], bf16, name=f"vt{b}")
            nc.scalar.dma_start(out=vtile[:], in_=vt[b][:])
            vt_sb.append(vtile)

        for nt in range(4):
            w = min(512, NQKV - nt * 512)
            nc.scalar.copy(qkv_sb[:, nt * 512: nt * 512 + w], qkv_ps[nt][:])

        # ---- transpose q into masked stationaries, k into kT ----
        kT_r = kT_sb[:].rearrange("p (b five) -> p b five", b=B, five=HPC)
        qTm_r = qTm_sb[:, :420].rearrange("p (b rest) -> p b rest", b=B, rest=105)
        for h in range(HPC):
            tq = psum.tile([128, B], bf16, tag="ps", name=f"tq_{h}")
            nc.tensor.transpose(
                tq[:], qkv_sb[:, h * 128:(h + 1) * 128], ident_sb[:B, :B]
            )
            nc.vector.tensor_copy(qTm_r[:, :, 21 * h], tq[:])
        for h in range(HPC):
            tk = psum.tile([128, B], bf16, tag="ps", name=f"tk_{h}")
            nc.tensor.transpose(
                tk[:], qkv_sb[:, EPC + h * 128: EPC + (h + 1) * 128],
                ident_sb[:B, :B],
            )
            nc.vector.tensor_copy(kT_r[:, :, h], tk[:])

        # ---- scatter new-token K column / V row ----
        for r in range(R):
            b = r // HPC
            ntp = pos[b] // 512
            col = (off[ntp] + live[ntp].index(r)) * 512 + pos[b] % 512
            nc.vector.tensor_copy(
                kt_sb[:, col:col + 1], kT_sb[:, r:r + 1]
            )
        for b in range(B):
            nc.gpsimd.dma_start(
                out=vt_sb[b][pos[b] % 128: pos[b] % 128 + 1, pos[b] // 128, :],
                in_=qkv_sb[b:b + 1, 2 * EPC:3 * EPC],
            )

        # ---- scores + softmax on [20, 512] tiles ----
        for nt in range(4):
            lo = nt * 512
            sp = psum.tile([R, 512], f32, tag="ps", name=f"sp{nt}")
            for i, r in enumerate(live[nt]):
                nc.tensor.matmul(
                    sp[:],
                    lhsT=qTm_sb[:, r * R:(r + 1) * R],
                    rhs=kt_sb[:, (off[nt] + i) * 512:(off[nt] + i + 1) * 512],
                    start=(i == 0),
                    stop=(i == len(live[nt]) - 1),
                )
            sf = sfpool.tile([R, 512], f32, tag="sf")
            nc.vector.tensor_add(sf[:], sp[:], term1_sb[:, lo: lo + 512])
            nc.scalar.activation(
                attn_sb[:, lo: lo + 512],
                sf[:],
                func=mybir.ActivationFunctionType.Exp,
                bias=term2_sb[:],
                accum_out=sums_sb[:, nt:nt + 1],
            )

        # ---- transpose attn chunks: [20, 128] -> [128, 20] ----
        ncmax = max(nch)
        for c in range(ncmax):
            ta = psum.tile([128, R], bf16, tag="ps", name=f"ta{c}")
            nc.tensor.transpose(ta[:], attn_sb[:, c * 128:(c + 1) * 128], ident_sb[:])
            nc.vector.tensor_copy(attnT_sb[:, c * R:(c + 1) * R], ta[:])

        # ---- softmax denominators -> per-sequence [5,1] recip tiles ----
        nc.vector.tensor_add(sum2_sb[:, 0:1], sums_sb[:, 0:1], sums_sb[:, 1:2])
        nc.vector.tensor_add(sum2_sb[:, 1:2], sums_sb[:, 2:3], sums_sb[:, 3:4])
        nc.vector.tensor_add(sumt_sb[:], sum2_sb[:, 0:1], sum2_sb[:, 1:2])
        nc.vector.reciprocal(recip_sb[:], sumt_sb[:])
        rr = psum.tile([1, R], f32, tag="ps", name="rr")
        nc.tensor.transpose(rr[:], recip_sb[:], identf_sb[:])
        nc.vector.tensor_copy(recip_row[:], rr[:])
        for b in range(B):
            rb = psum.tile([HPC, 1], f32, tag="ps", name=f"rb{b}")
            nc.tensor.transpose(
                rb[:], recip_row[:, b * HPC:(b + 1) * HPC], identf_sb[:1, :1]
            )
            nc.vector.tensor_copy(recip_b[b][:], rb[:])

        # ---- attn @ V, batched over the 5 heads of each sequence ----
        for b in range(B):
            ao0 = psum.tile([HPC, 512], f32, tag="ps", name=f"ao0_{b}")
            ao1 = psum.tile([HPC, EPC - 512], f32, tag="ps", name=f"ao1_{b}")
            for c in range(nch[b]):
                lt = attnT_sb[:, c * R + b * HPC: c * R + (b + 1) * HPC]
                nc.tensor.matmul(
                    ao0[:], lhsT=lt, rhs=vt_sb[b][:, c, 0:512],
                    start=(c == 0), stop=(c == nch[b] - 1),
                )
                nc.tensor.matmul(
                    ao1[:], lhsT=lt, rhs=vt_sb[b][:, c, 512:EPC],
                    start=(c == 0), stop=(c == nch[b] - 1),
                )
            nc.scalar.activation(
                ao_sb[b][:, 0:512], ao0[:],
                func=mybir.ActivationFunctionType.Copy, scale=recip_b[b][:],
            )
            nc.scalar.activation(
                ao_sb[b][:, 512:EPC], ao1[:],
                func=mybir.ActivationFunctionType.Copy, scale=recip_b[b][:],
            )

        # ---- transpose attn_out diag blocks -> aoT [128, 20] (col h*B+b) ----
        for b in range(B):
            for h in range(HPC):
                to = psum.tile([128, HPC], bf16, tag="ps", name=f"to{b}_{h}")
                nc.tensor.transpose(
                    to[:], ao_sb[b][:, h * 128:(h + 1) * 128], ident_sb[:HPC, :HPC]
                )
                nc.vector.tensor_copy(
                    aoT_sb[:, h * B + b: h * B + b + 1], to[:, h:h + 1]
                )

        # ---- output projection: out[4, 5120]; wo streams on sync ring ----
        for jg in range(10):
            wt = wopool.tile([128, HPC * 512], bf16, tag="wo", name=f"wo{jg}")
            nc.sync.dma_start(out=wt[:], in_=wo[:, :, jg * 512:(jg + 1) * 512])
            op = psum.tile([B, 512], f32, tag="ps", name=f"op{jg}")
            for hc in range(HPC):
                nc.tensor.matmul(
                    op[:],
                    lhsT=aoT_sb[:, hc * B:(hc + 1) * B],
                    rhs=wt[:, hc * 512:(hc + 1) * 512],
                    start=(hc == 0),
                    stop=(hc == HPC - 1),
                )
            nc.scalar.copy(out_sb[:, jg * 512:(jg + 1) * 512], op[:])

        nc.sync.dma_start(out=outp[:], in_=out_sb[:])

    nc.compile()
    return nc


def _bf16(x):
    return np.ascontiguousarray(x.astype(BF16))


def _prepare_core_inputs(core, hT_full, qkv_bf, o_bf, k_bf, v_bf, bt, sl, pos,
                         ident, identf, slopes_all):
    hs = slice(core * HPC, (core + 1) * HPC)
    es = slice(core * EPC, (core + 1) * EPC)
    live, off, nlive = _kt_sections(sl)
    nch = [(s + 127) // 128 for s in sl]

    # Wcat [128, 40, 1920]: Wcat[p, kc, j] = W[kc*128+p, j]; q pre-scaled.
    wcat = np.concatenate(
        [qkv_bf[0][:, es], qkv_bf[1][:, es], qkv_bf[2][:, es]], axis=1
    )  # [5120, 1920] bf16
    wcat = np.ascontiguousarray(wcat.reshape(NKC, 128, NQKV).transpose(1, 0, 2))

    # K^T per sequence/head: ktb[b][d, h, t]
    kg = k_bf[:, hs]   # [NB, 5, 16, 128] bf16
    ktb = []
    for b in range(B):
        kk = kg[bt[b]].transpose(1, 0, 2, 3).reshape(HPC, S, D)
        ktb.append(kk.transpose(2, 0, 1))           # [d, h, t]
    # packed live-section image [128, nlive*512]
    kt = np.empty((128, nlive * 512), BF16)
    for nt in range(4):
        for i, r in enumerate(live[nt]):
            b, h = divmod(r, HPC)
            c0 = (off[nt] + i) * 512
            kt[:, c0:c0 + 512] = ktb[b][:, h, nt * 512:(nt + 1) * 512]

    # vt[b] [128(t%128), nch, 640(h,d)] truncated
    vg = v_bf[:, hs]
    vts = {}
    for b in range(B):
        vv = vg[bt[b]].transpose(0, 2, 1, 3).reshape(S, HPC, D)   # [t, h, d]
        vts[f"vt{b}"] = np.ascontiguousarray(
            vv.reshape(16, 128, HPC * D).transpose(1, 0, 2)[:, :nch[b], :]
        )

    # wo [128, 5, 5120]: wo[p, h, j] = Wo[h*128+p, j]
    wo = np.ascontiguousarray(o_bf[es, :].reshape(HPC, 128, E).transpose(1, 0, 2))

    # alibi: term1[r, t] = slope_h * t (masked), term2[r] = -slope_h*pos_b
    slopes = slopes_all[core * HPC:(core + 1) * HPC]
    t_idx = np.arange(S, dtype=np.float32)
    term1 = np.empty((B, HPC, S), np.float32)
    term2 = np.empty((B, HPC, 1), np.float32)
    for b in range(B):
        term1[b] = slopes[:, None] * t_idx[None, :]
        term1[b, :, sl[b]:] = NEG
        term2[b, :, 0] = -slopes * np.float32(pos[b])

    return dict(hT=hT_full, wcat=wcat, kt=kt, wo=wo,
                term1=np.ascontiguousarray(term1.reshape(R, S)),
                term2=np.ascontiguousarray(term2.reshape(R, 1)),
                ident=ident, identf=identf, **vts)


def kernel(**inputs):
    global LAST_RESULTS
    hidden = np.asarray(inputs["hidden_states"], np.float32)
    qkv_w = np.asarray(inputs["qkv_weight"], np.float32)
    o_w = np.asarray(inputs["o_proj_weight"], np.float32)
    k_cache = np.asarray(inputs["k_cache"], np.float32)
    v_cache = np.asarray(inputs["v_cache"], np.float32)
    bt = np.asarray(inputs["block_tables"]).astype(np.int64)
    sl = np.asarray(inputs["sequence_lengths"]).astype(np.int64)

    pos = tuple(int(x) - 1 for x in sl)
    slt = tuple(int(x) for x in sl)

    # Shared host-side conversions (bf16 once, slice per core after).
    qkv_bf = [
        _bf16(qkv_w[0] * np.float32(D ** -0.5)),
        _bf16(qkv_w[1]),
        _bf16(qkv_w[2]),
    ]
    o_bf = _bf16(o_w)
    k_bf = _bf16(k_cache)
    v_bf = _bf16(v_cache)
    hT_full = _bf16(
        hidden.T.reshape(NKC, 128, B).transpose(1, 0, 2).reshape(128, NKC * B)
    )
    ident = np.eye(20, dtype=BF16)
    identf = np.eye(20, dtype=np.float32)
    slopes_all = _alibi_slopes(H)

    in_maps = [
        _prepare_core_inputs(c, hT_full, qkv_bf, o_bf, k_bf, v_bf, bt, slt, pos,
                             ident, identf, slopes_all)
        for c in range(NCORES)
    ]

    if pos not in _PROGRAM_CACHE:
        _PROGRAM_CACHE[pos] = _build_program(pos)
    nc = _PROGRAM_CACHE[pos]

    from concourse.bass_utils import run_bass_kernel_spmd

    res = run_bass_kernel_spmd(
        nc,
        in_maps,
        core_ids=list(range(NCORES)),
        trace=bool(os.environ.get("BASS_TRACE")),
    )
    LAST_RESULTS = res

    out = np.zeros((B, E), np.float64)
    for c in range(NCORES):
        out += np.asarray(res.results[c]["outp"]).astype(np.float64)
    return out.astype(np.float32)
